# revision 82
# baseline (speedup 1.0000x reference)
"""MultiHeadLatentAttention (MLA) Trainium2 Bass kernel, v2.

Problem: B=2, S=2048, D=2048, H=16 heads, d_nope=128, d_rope=64, d_head=128,
q_latent=768, kv_latent=512. Causal attention, rmsnorm'd latents, half-dim RoPE.

Sharding (8 cores): core c handles batch b=c//4 and head group g=c%4 (4 heads).
The small latent down-projections are replicated within each batch group;
W_uq/W_qr/W_uk/W_kr/W_uv are column-sharded by head; W_o row-sharded; the
4 partial outputs per batch are summed on the host.

Precision/engine plan (metric = InstructionCostModel timeline; baseline
559461 ns -> 276691 ns, rel err 1.32e-2 < 2e-2):
  - scores matmul in fp8e4m3 with MatmulPerfMode.DoubleRow: the nope(128) and
    zero-padded rope(64) contractions are packed as the two DoubleRow k-tiles,
    so each 128x512 score tile costs 256 PE cycles instead of 1024.
    End-to-end error from quantizing qn/kn/qr/kr to fp8 measured 1.33e-2;
    every other fp8 stage below is a lossless-ish residual split adding <1e-3.
  - causal mask added in the same PSUM group by a fp8 DoubleRow matmul of
    60*I against a {0,-240} mask table (-14400 pre-scale -> exp()=2e-9).
  - all projections (down, up, W_o) run as 3-term fp8 DoubleRow residual
    splits (W8@x8 + W8r@x8 + W8@x8r, dropping the second-order W8r@x8r):
    fp8 PE speed (0.5 cycles/row) at fp16-like accuracy. Weights are split
    on the host; x arrives pre-split; latents are split once at the rmsnorm
    multiply; attention outputs are split on DVE before W_o.
  - kv down-projection is S-sharded: each core computes only its own 512-col
    block and the fp8 hi+lo latent pair is AllGathered (2.1MB) on the
    collective cores, overlapped with the replicated q down-projection.
    The collective's SWDGE upload/downloads live on the otherwise-empty Pool
    queue so its in-order waits block nothing.
  - PV in fp16 (exp quantization to fp8 would cost ~3% output error).
  - softmax: exp on ACT (fp16 out), denominator via two alternating fp16
    accumulators on DVE + partition_all_reduce on Pool + DVE reciprocal --
    no Ln anywhere, so a single act-table load (was 49 reloads/63us).
  - rmsnorm rsqrt = ACT Sqrt(DVE reciprocal(mean sq)); the partition
    broadcast of the per-token scale is a PE outer product (ones x row) so
    P0 keeps the Pool queue empty for the collective.
  - W_o, latents, k^T, v stay resident in SBUF; W_o(prev block) d-tiles and
    q8(next block) are interleaved into the in-order PE stream as fill work
    during exp-latency stalls; yT stores pair two d-tiles per DMA.
"""
import math
import os
from contextlib import ExitStack

import numpy as np
import ml_dtypes

import concourse.bass as bass
import concourse.bass_isa as bass_isa
import concourse.bacc as bacc
import concourse.mybir as mybir
import concourse.tile as tile
from concourse.bass_utils import run_bass_kernel_spmd

F32 = mybir.dt.float32
F32R = mybir.dt.float32r
F16 = mybir.dt.float16
F8 = mybir.dt.float8e4
AF = mybir.ActivationFunctionType
DR_MODE = mybir.MatmulPerfMode.DoubleRow

B, S_FULL, D = 2, 2048, 2048
H, DN, DRR, DH = 16, 128, 64, 128
QL, KVL = 768, 512
EPS = 1e-6
SCALE = 1.0 / math.sqrt(DH)
NCORES = 8
NKT = D // 128          # 16 contraction tiles over D
NKP = NKT // 2          # 8 DoubleRow pairs
NLQ = QL // 128         # 6
NLKV = KVL // 128       # 4
NDT = D // 128          # 16 output D tiles

# fp8 scaling for the residual-split down-projection: x' = x*AX, W' = W*BW
# so both the quantized tensors and their residuals stay in fp8 normal range.
AX = 32.0
BW = 256.0
PSUM_UNSCALE = 1.0 / (AX * BW)
# residual-split up-projections: latents x SL (folded into the rsqrt), and
# the up-projection weights x BW
SL = 16.0
UP_UNSCALE = 1.0 / (SL * BW)
# score operand quantization scale (qn8 = 8*qn etc.)
SQ8 = 8.0
EXP_SCALE = SCALE / (SQ8 * SQ8)
MASK_ID = 60.0          # mask matmul: 60 * (-240) * 1 plane = -14400 pre-scale
# W_o fp8 residual split: out tiles scaled x32 (folded into v), W_o x1024
SO = 32.0
BWO = 1024.0
Y_UNSCALE = 1.0 / (SO * BWO)

PHASE_MARKS = {}


def build_nc(S=S_FULL):
    assert S % 512 == 0
    n_sb = S // 512
    n_st = S // 128
    PHASE_MARKS.clear()

    nc = bacc.Bacc("TRN2", target_bir_lowering=False, debug=False,
                   num_devices=NCORES)

    # x arrives pre-packed in device layout: per 512-col block, partition p
    # holds its 16 contraction tiles contiguously (1 DMA segment/partition)
    x8_d = nc.dram_tensor("x8", [S // 512, 128, NKT * 512], F8,
                          kind="ExternalInput")
    x8r_d = nc.dram_tensor("x8r", [S // 512, 128, NKT * 512], F8,
                           kind="ExternalInput")
    xkv8_d = nc.dram_tensor("xkv8", [128, NKT * 512], F8,
                            kind="ExternalInput")
    xkv8r_d = nc.dram_tensor("xkv8r", [128, NKT * 512], F8,
                             kind="ExternalInput")
    wdq_d = nc.dram_tensor("W_dq8", [128, NKT * QL], F8,
                           kind="ExternalInput")
    wdqr_d = nc.dram_tensor("W_dq8r", [128, NKT * QL], F8,
                            kind="ExternalInput")
    wdkv_d = nc.dram_tensor("W_dkv8", [128, NKT * KVL], F8,
                            kind="ExternalInput")
    wdkvr_d = nc.dram_tensor("W_dkv8r", [128, NKT * KVL], F8,
                             kind="ExternalInput")
    wuq_d = nc.dram_tensor("Wuq8", [QL, 512], F8, kind="ExternalInput")
    wuqr_d = nc.dram_tensor("Wuq8r", [QL, 512], F8, kind="ExternalInput")
    wqr_d = nc.dram_tensor("Wqr8", [QL, 256], F8, kind="ExternalInput")
    wqrr_d = nc.dram_tensor("Wqr8r", [QL, 256], F8, kind="ExternalInput")
    wuk_d = nc.dram_tensor("Wuk8", [KVL, 512], F8, kind="ExternalInput")
    wukr_d = nc.dram_tensor("Wuk8r", [KVL, 512], F8, kind="ExternalInput")
    wkr_d = nc.dram_tensor("Wkr8", [KVL, 256], F8, kind="ExternalInput")
    wkrr_d = nc.dram_tensor("Wkr8r", [KVL, 256], F8, kind="ExternalInput")
    wuv_d = nc.dram_tensor("Wuv8", [KVL, 512], F8, kind="ExternalInput")
    wuvr_d = nc.dram_tensor("Wuv8r", [KVL, 512], F8, kind="ExternalInput")
    wo_d = nc.dram_tensor("Wo8", [512, D], F8, kind="ExternalInput")
    wor_d = nc.dram_tensor("Wo8r", [512, D], F8, kind="ExternalInput")
    c4_d = nc.dram_tensor("c4", [128, S], F16, kind="ExternalInput")
    s4_d = nc.dram_tensor("s4", [128, S], F16, kind="ExternalInput")
    mask_d = nc.dram_tensor("mask8", [128, 2, 896], F8, kind="ExternalInput")
    id_d = nc.dram_tensor("id8", [128, 2, 128], F8, kind="ExternalInput")
    ones_d = nc.dram_tensor("ones8", [128, 1], F8, kind="ExternalInput")
    yT_d = nc.dram_tensor("yT", [D, S], F16, kind="ExternalOutput")
    debug = bool(int(os.environ.get("MLA_DEBUG", "0")))
    if debug:
        dbg_qlat_d = nc.dram_tensor("dbg_qlat", [128, NLQ, S], F16,
                                    kind="ExternalOutput")
        dbg_kvlat_d = nc.dram_tensor("dbg_kvlat", [128, NLKV, S], F16,
                                     kind="ExternalOutput")
        dbg_kT8_d = nc.dram_tensor("dbg_kT8", [128, 4, 2, S], F16,
                                   kind="ExternalOutput")
        dbg_v_d = nc.dram_tensor("dbg_v", [128, n_st, 512], F16,
                                 kind="ExternalOutput")

    def col3(dram_ap, p=128):
        # [R, C] dram slice -> [128, R//128, C] tiled AP
        return dram_ap.rearrange("(t p) c -> p t c", p=p)

    def rope_pair(nc, pool, outs, ps, c4s, s4s):
        """Half-dim rope on a 2-head pair tile [128, 512] in PSUM.

        out = ps * c4 + shuf(ps) * s4, shuf swaps 32-blocks within each 64.
        Stages through fp16 SBUF so the DVE muls run in 2x mode. `outs` is a
        list of (out_ap, pslice) fp8 destinations.
        """
        rs = pool.tile([128, 512], F16, tag="rope_rs")
        nc.scalar.activation(rs[:], ps[:], AF.Copy, scale=UP_UNSCALE)
        shuf = pool.tile([128, 512], F16, tag="rope_shuf")
        nc.vector.tensor_copy(shuf[0:32, :], rs[32:64, :])
        nc.vector.tensor_copy(shuf[32:64, :], rs[0:32, :])
        nc.vector.tensor_copy(shuf[64:96, :], rs[96:128, :])
        nc.vector.tensor_copy(shuf[96:128, :], rs[64:96, :])
        t1 = pool.tile([128, 512], F16, tag="rope_t1")
        nc.vector.tensor_mul(t1[:], rs[:], c4s)
        nc.vector.tensor_mul(shuf[:], shuf[:], s4s)
        for out_ap, psl in outs:
            nc.vector.tensor_add(out_ap, t1[psl, :], shuf[psl, :])

    with tile.TileContext(nc) as tc:
        with (
            tc.tile_pool(name="const", bufs=1) as constp,
            tc.tile_pool(name="ps_mm", bufs=2, space="PSUM") as ps_mm,
            tc.tile_pool(name="ps_o", bufs=2, space="PSUM") as ps_op,
        ):
            def alt_ps(i):
                if i % 2 == 0:
                    return ps_mm.tile([128, 512], F32, tag="mm", name="ps")
                return ps_op.tile([128, 512], F32, tag="pv", name="ps")

            mask_t = constp.tile([128, 2, 896], F8)
            id_t = constp.tile([128, 2, 128], F8)
            ones_t = constp.tile([128, 1], F8)
            ones_row = constp.tile([1, 128], F16)
            nc.vector.memset(ones_row[:], 1.0)

            # persistent SBUF state
            persist_stack = ExitStack()
            persist = persist_stack.enter_context(
                tc.tile_pool(name="persist", bufs=1))
            # kT8: per head (rope_padded, nope) planes, fp8 stationary
            kT8_t = persist.tile([128, 4, 2, S], F8)
            v_t = persist.tile([128, n_st, 512], F16)
            wo_t = persist.tile([128, 4, D], F8)
            wor_t = persist.tile([128, 4, D], F8)
            qlat_t = persist.tile([128, NLQ, S], F8)
            qlatr_t = persist.tile([128, NLQ, S], F8)
            kvlat_t = persist.tile([128, NLKV, S], F8)
            kvlatr_t = persist.tile([128, NLKV, S], F8)

            # ---------------- P0: down-projections + rmsnorm ----------------
            PHASE_MARKS["P0"] = nc.next_id()
            p0_stack = ExitStack()
            p0w = p0_stack.enter_context(tc.tile_pool(name="p0w", bufs=1))
            wdq_t = p0w.tile([128, NKT, QL], F8)
            wdqr_t = p0w.tile([128, NKT, QL], F8)
            wdkv_t = p0w.tile([128, NKT, KVL], F8)
            wdkvr_t = p0w.tile([128, NKT, KVL], F8)
            with (
                tc.tile_pool(name="p0x", bufs=2) as p0x,
                tc.tile_pool(name="p0raw", bufs=2) as p0raw,
                tc.tile_pool(name="p0sq", bufs=2) as p0sq,
                tc.tile_pool(name="p0own", bufs=1) as p0own,
                tc.tile_pool(name="p0tmp", bufs=2) as p0tmp,
                tc.tile_pool(name="p0dram", bufs=1, space="DRAM") as p0dram,
                tc.tile_pool(name="ps_den", bufs=1, space="PSUM") as ps_denp,
                tc.tile_pool(name="ps_p0", bufs=2, space="PSUM") as ps_p0,
            ):
                def alt3_ps(i):
                    if i % 3 == 2:
                        return ps_p0.tile([128, 512], F32, tag="p0", name="ps")
                    return alt_ps(i % 3)

                # zero the pad halves of the rope planes once (rope data for
                # even heads lives at partitions 0:64, odd heads at 64:128)
                for h in range(4):
                    lo = 0 if h % 2 else 64
                    nc.gpsimd.memset(kT8_t[lo:lo + 64, h, 0, :], 0.0)

                def down_proj(latname, w_t, wr_t, nl, xh, xrh, dest8,
                              dest8r, rawp=None, sqp=None, terms3=True):
                    raw = (rawp or p0raw).tile(
                        [128, nl, 512], F16, tag=f"raw{latname}",
                        name=f"raw{latname}")
                    sq = (sqp or p0sq).tile(
                        [128, nl, 512], F8, tag=f"sq{latname}",
                        name=f"sq{latname}")
                    ps_ss = ps_denp.tile([1, 512], F32, tag="den")
                    for lt in range(nl):
                        ps = alt3_ps(lt)
                        lsl = slice(128 * lt, 128 * (lt + 1))
                        terms = ((w_t, xh), (wr_t, xh), (w_t, xrh)) \
                            if terms3 else ((w_t, xh), (w_t, xrh))
                        for term_w, term_x in terms:
                            first = term_w is w_t and term_x is xh
                            last = term_x is xrh
                            for kp in range(NKP):
                                nc.tensor.matmul(
                                    ps[:],
                                    term_w[:, 2 * kp:2 * kp + 2, lsl],
                                    term_x[:, 2 * kp:2 * kp + 2, :],
                                    start=(first and kp == 0),
                                    stop=(last and kp == NKP - 1),
                                    perf_mode=DR_MODE,
                                )
                        nc.scalar.activation(
                            raw[:, lt, :], ps[:], AF.Copy, scale=PSUM_UNSCALE)
                        nc.scalar.activation(
                            sq[:, lt, :], ps[:], AF.Square, scale=PSUM_UNSCALE)
                        nc.tensor.matmul(
                            ps_ss[:], ones_t[:], sq[:, lt, :],
                            start=(lt == 0), stop=(lt == nl - 1),
                        )
                    # rsqrt of mean square: sqrt(1/m) via DVE reciprocal
                    # + ACT Sqrt (Copy/Square live in the sqrt act table,
                    # so P0 needs no act-table reloads)
                    mrow = p0tmp.tile([1, 512], F32, tag="mrow")
                    nc.vector.tensor_scalar(
                        mrow[:], ps_ss[:], 1.0 / (128 * nl), EPS,
                        mybir.AluOpType.mult, mybir.AluOpType.add)
                    rrec = p0tmp.tile([1, 512], F32, tag="rrec")
                    nc.vector.reciprocal(rrec[:], mrow[:])
                    rrow = p0tmp.tile([1, 512], F16, tag="rrow")
                    # scale=SL^2 folds the latent fp8 scale into the rsqrt
                    nc.scalar.activation(rrow[:], rrec[:], AF.Sqrt,
                                         scale=SL * SL)
                    # broadcast across partitions via a PE outer product so P0
                    # keeps the Pool queue empty (the AllGather blocks it)
                    ps_bc = ps_denp.tile([128, 512], F32, tag="bc")
                    nc.tensor.matmul(ps_bc[:], ones_row[:], rrow[:],
                                     start=True, stop=True)
                    rsb = p0tmp.tile([128, 512], F16, tag="rsb")
                    nc.scalar.copy(rsb[:], ps_bc[:])
                    for lt in range(nl):
                        tmp = p0tmp.tile([128, 512], F16, tag="ntmp")
                        nc.vector.tensor_mul(tmp[:], raw[:, lt, :], rsb[:])
                        nc.vector.tensor_copy(dest8(lt), tmp[:])
                        nc.vector.tensor_sub(dest8r(lt), tmp[:], dest8(lt))

                # --- kv down-proj for this core's own block only; the other
                # blocks arrive via an AllGather of the fp16 latents that
                # overlaps with the (replicated) q down-projection.
                xkvh = p0x.tile([128, NKT, 512], F8, tag="x8", name="xkv8")
                xkvrh = p0x.tile([128, NKT, 512], F8, tag="x8r", name="xkv8r")
                nc.sync.dma_start(xkvh[:], xkv8_d[:])
                nc.sync.dma_start(ones_t[:], ones_d[:])
                nc.sync.dma_start(wdkv_t[:], wdkv_d[:])
                nc.sync.dma_start(wdkvr_t[:], wdkvr_d[:])
                nc.sync.dma_start(xkvrh[:], xkv8r_d[:])
                kvlat_own = p0own.tile([128, NLKV, 512], F8, tag="kvown",
                                       name="kvlat_own")
                kvlatr_own = p0own.tile([128, NLKV, 512], F8, tag="kvownr",
                                        name="kvlatr_own")
                down_proj("kv", wdkv_t, wdkvr_t, NLKV, xkvh, xkvrh,
                          lambda lt: kvlat_own[:, lt, :],
                          lambda lt: kvlatr_own[:, lt, :],
                          rawp=p0own, sqp=p0own)
                # the whole collective path lives on the (otherwise idle)
                # Pool queue: its in-order waits must not block the SP/ACT
                # DMA queues or the ACT compute stream
                kv_own_d = p0dram.tile([2 * KVL, 512], F8, name="kv_own")
                kv_all_d = p0dram.tile([8 * KVL, 512], F8, name="kv_all")
                nc.gpsimd.dma_start(col3(kv_own_d[0:KVL, :]), kvlat_own[:])
                nc.gpsimd.dma_start(col3(kv_own_d[KVL:2 * KVL, :]),
                                    kvlatr_own[:])
                nc.gpsimd.collective_compute(
                    "AllGather",
                    mybir.AluOpType.bypass,
                    replica_groups=[[0, 1, 2, 3], [4, 5, 6, 7]],
                    ins=[kv_own_d[:]],
                    outs=[kv_all_d[:]],
                )
                for c in range(4):
                    base = c * 2 * KVL
                    nc.gpsimd.dma_start(
                        kvlat_t[:, :, 512 * c:512 * (c + 1)],
                        col3(kv_all_d[base:base + KVL, :]))
                    nc.gpsimd.dma_start(
                        kvlatr_t[:, :, 512 * c:512 * (c + 1)],
                        col3(kv_all_d[base + KVL:base + 2 * KVL, :]))

                # --- replicated q down-projection over all blocks
                for sb in range(n_sb):
                    cs = slice(512 * sb, 512 * (sb + 1))
                    xh = p0x.tile([128, NKT, 512], F8, tag="x8", name=f"x8_{sb}")
                    xrh = p0x.tile([128, NKT, 512], F8, tag="x8r",
                                   name=f"x8r_{sb}")
                    nc.sync.dma_start(xh[:], x8_d[sb])
                    if sb == 0:
                        nc.sync.dma_start(wdq_t[:], wdq_d[:])
                        nc.sync.dma_start(wdqr_t[:], wdqr_d[:])
                    nc.sync.dma_start(xrh[:], x8r_d[sb])
                    if sb == 0:
                        nc.sync.dma_start(mask_t[:], mask_d[:])
                        nc.sync.dma_start(id_t[:], id_d[:])
                    if sb == 3:
                        # W_o resident load (needed only from P2, and after
                        # the last x chunks so it never delays them)
                        nc.sync.dma_start(wo_t[:], col3(wo_d[:]))
                        nc.sync.dma_start(wor_t[:], col3(wor_d[:]))
                    down_proj("q", wdq_t, wdqr_t, NLQ, xh, xrh,
                              lambda lt, cs=cs: qlat_t[:, lt, cs],
                              lambda lt, cs=cs: qlatr_t[:, lt, cs])
            p0_stack.close()

            # ---------------- P1: k/v up-projections ----------------
            PHASE_MARKS["P1"] = nc.next_id()
            p2w_stack = ExitStack()
            p2w = p2w_stack.enter_context(tc.tile_pool(name="p2w", bufs=1))
            p2q = p2w_stack.enter_context(tc.tile_pool(name="p2q", bufs=2))
            p2tmp = p2w_stack.enter_context(tc.tile_pool(name="p2tmp", bufs=2))
            wuq_t = p2w.tile([128, NLQ, 512], F8)
            wuqr_t = p2w.tile([128, NLQ, 512], F8)
            wqr_t = p2w.tile([128, NLQ, 256], F8)
            wqrr_t = p2w.tile([128, NLQ, 256], F8)

            def compute_q8(qb):
                """q up-projection + rope for one q-block into a fp8 moving
                tile with slots (qr01, qn0, qn1, qr23, qn2, qn3)."""
                cs = slice(512 * qb, 512 * (qb + 1))
                c4s = p2tmp.tile([128, 512], F16, tag="c4")
                s4s = p2tmp.tile([128, 512], F16, tag="s4")
                nc.sync.dma_start(c4s[:], c4_d[:, cs])
                nc.sync.dma_start(s4s[:], s4_d[:, cs])
                q8 = p2q.tile([128, 6, 512], F8, tag="q8", name=f"q8_{qb}")

                def up_chain(ps, w_t, wr_t, ccols, np_, lat=qlat_t,
                             latr=qlatr_t):
                    terms = ((w_t, lat), (wr_t, lat), (w_t, latr))
                    for ti, (tw, tl) in enumerate(terms):
                        for qp in range(np_):
                            nc.tensor.matmul(
                                ps[:], tw[:, 2 * qp:2 * qp + 2, ccols],
                                tl[:, 2 * qp:2 * qp + 2, cs],
                                start=(ti == 0 and qp == 0),
                                stop=(ti == 2 and qp == np_ - 1),
                                perf_mode=DR_MODE,
                            )

                for h in range(4):
                    ps = alt_ps(h)
                    up_chain(ps, wuq_t, wuqr_t,
                             slice(128 * h, 128 * (h + 1)), NLQ // 2)
                    slot = (1, 2, 4, 5)[h]
                    nc.scalar.activation(
                        q8[:, slot, :], ps[:], AF.Copy,
                        scale=SQ8 * UP_UNSCALE)
                for pr in range(2):
                    ps = alt_ps(pr)
                    up_chain(ps, wqr_t, wqrr_t,
                             slice(128 * pr, 128 * (pr + 1)), NLQ // 2)
                    rope_pair(nc, p2tmp,
                              [(q8[:, 3 * pr, :], slice(0, 128))],
                              ps, c4s[:], s4s[:])
                return q8

            with (
                tc.tile_pool(name="p1w", bufs=1) as p1w,
                tc.tile_pool(name="p1tmp", bufs=2) as p1tmp,
            ):
                wuk_t = p1w.tile([128, NLKV, 512], F8)
                wukr_t = p1w.tile([128, NLKV, 512], F8)
                wkr_t = p1w.tile([128, NLKV, 256], F8)
                wkrr_t = p1w.tile([128, NLKV, 256], F8)
                wuv_t = p1w.tile([128, NLKV, 512], F8)
                wuvr_t = p1w.tile([128, NLKV, 512], F8)
                nc.sync.dma_start(wuk_t[:], col3(wuk_d[:]))
                nc.sync.dma_start(wukr_t[:], col3(wukr_d[:]))
                nc.sync.dma_start(wuq_t[:], col3(wuq_d[:]))
                nc.sync.dma_start(wuqr_t[:], col3(wuqr_d[:]))
                nc.sync.dma_start(wqr_t[:], col3(wqr_d[:]))
                nc.sync.dma_start(wqrr_t[:], col3(wqrr_d[:]))
                # q8 for blocks 0 and 1 first: their inputs are ready
                # before the AllGathered kv latents land, filling the P1
                # entry stall with ~15us of PE work
                q8_0 = compute_q8(0)
                q8_1 = compute_q8(1)
                for sb in range(n_sb):
                    cs = slice(512 * sb, 512 * (sb + 1))
                    if sb == 0:
                        nc.sync.dma_start(wkr_t[:], col3(wkr_d[:]))
                        nc.sync.dma_start(wkrr_t[:], col3(wkrr_d[:]))
                        nc.sync.dma_start(wuv_t[:], col3(wuv_d[:]))
                        nc.sync.dma_start(wuvr_t[:], col3(wuvr_d[:]))
                    c4s = p1tmp.tile([128, 512], F16, tag="c4")
                    s4s = p1tmp.tile([128, 512], F16, tag="s4")
                    nc.sync.dma_start(c4s[:], c4_d[:, cs])
                    nc.sync.dma_start(s4s[:], s4_d[:, cs])
                    def kv_chain(ps, w_t, wr_t, ccols):
                        terms = ((w_t, kvlat_t), (wr_t, kvlat_t),
                                 (w_t, kvlatr_t))
                        for ti, (tw, tl) in enumerate(terms):
                            for kp in range(NLKV // 2):
                                nc.tensor.matmul(
                                    ps[:], tw[:, 2 * kp:2 * kp + 2, ccols],
                                    tl[:, 2 * kp:2 * kp + 2, cs],
                                    start=(ti == 0 and kp == 0),
                                    stop=(ti == 2 and kp == NLKV // 2 - 1),
                                    perf_mode=DR_MODE,
                                )

                    for h in range(4):
                        ps = alt_ps(h)
                        kv_chain(ps, wuk_t, wukr_t,
                                 slice(128 * h, 128 * (h + 1)))
                        nc.scalar.activation(
                            kT8_t[:, h, 1, cs], ps[:], AF.Copy,
                            scale=SQ8 * UP_UNSCALE)
                    for pr in range(2):
                        ps = alt_ps(pr)
                        kv_chain(ps, wkr_t, wkrr_t,
                                 slice(128 * pr, 128 * (pr + 1)))
                        he, ho = 2 * pr, 2 * pr + 1
                        rope_pair(
                            nc, p1tmp, [
                                (kT8_t[0:64, he, 0, cs], slice(0, 64)),
                                (kT8_t[64:128, ho, 0, cs], slice(64, 128)),
                            ], ps, c4s[:], s4s[:])
                    for stl in range(4):
                        st = 4 * sb + stl
                        stc = slice(512 * sb + 128 * stl,
                                    512 * sb + 128 * (stl + 1))
                        ps = alt_ps(stl)
                        terms = ((kvlat_t, wuv_t), (kvlatr_t, wuv_t),
                                 (kvlat_t, wuvr_t))
                        for ti, (tl, tw) in enumerate(terms):
                            for kp in range(NLKV // 2):
                                nc.tensor.matmul(
                                    ps[:], tl[:, 2 * kp:2 * kp + 2, stc],
                                    tw[:, 2 * kp:2 * kp + 2, :],
                                    start=(ti == 0 and kp == 0),
                                    stop=(ti == 2 and kp == NLKV // 2 - 1),
                                    perf_mode=DR_MODE,
                                )
                        # x SO so the fp8 split of attention outputs uses
                        # fp8 normal range (unscaled at the yT stage)
                        nc.scalar.activation(v_t[:, st, :], ps[:], AF.Copy,
                                             scale=SO * UP_UNSCALE)

            if debug:
                nc.sync.dma_start(dbg_qlat_d[:], qlat_t[:])
                nc.sync.dma_start(dbg_kvlat_d[:], kvlat_t[:])
                nc.sync.dma_start(dbg_v_d[:], v_t[:])
                nc.gpsimd.dma_start(dbg_kT8_d[:], kT8_t[:])

            # ---------------- P2: attention + W_o ----------------
            PHASE_MARKS["P2"] = nc.next_id()
            with (
                tc.tile_pool(name="p2exp", bufs=5) as p2exp,
                tc.tile_pool(name="ps_s", bufs=2, space="PSUM") as ps_sp,
                tc.tile_pool(name="p2acc", bufs=2) as p2acc,
                tc.tile_pool(name="p2acc1", bufs=2) as p2acc1,
                tc.tile_pool(name="p2out", bufs=2) as p2out,
                tc.tile_pool(name="p2y", bufs=4) as p2y,
            ):
                def emit_wo(outs, cs, dts=range(NDT), alt=False):
                    o8, o8r = outs
                    ystage = None
                    for dt in dts:
                        dsl = slice(128 * dt, 128 * (dt + 1))
                        # the final (non-interleaved) call alternates PSUM
                        # pools for 4-bank pipelining against the ystage drain
                        ps_y = alt_ps(dt if alt else 0)
                        for j in (0, 1):
                            hp = slice(2 * j, 2 * j + 2)
                            for ti, (w_s, o_s) in enumerate(
                                ((wo_t, o8), (wor_t, o8), (wo_t, o8r))
                            ):
                                nc.tensor.matmul(
                                    ps_y[:], w_s[:, hp, dsl], o_s[:, hp, :],
                                    start=(j == 0 and ti == 0),
                                    stop=(j == 1 and ti == 2),
                                    perf_mode=DR_MODE,
                                )
                        # pair two d-tiles per ystage buffer and yT store to
                        # halve the store count (the rows are DRAM-adjacent)
                        if ystage is None:
                            ystage = p2y.tile([128, 2, 512], F16, tag="y")
                        half = ystage[:, dt % 2, :]
                        if dt % 2 == 0:
                            nc.vector.tensor_scalar(
                                half, ps_y[:], Y_UNSCALE, None,
                                mybir.AluOpType.mult)
                        else:
                            nc.scalar.activation(
                                half, ps_y[:], AF.Copy, scale=Y_UNSCALE)
                            nc.sync.dma_start(
                                col3(yT_d[128 * (dt - 1):128 * (dt + 1), cs]),
                                ystage[:])
                            ystage = None

                prev_out = None
                prev_cs = None
                q8_next = None
                for qb in range(n_sb):
                    cs = slice(512 * qb, 512 * (qb + 1))
                    q8 = (q8_0, q8_1)[qb] if qb < 2 else q8_next

                    def q8_mov(h):
                        base = 3 * (h // 2)
                        if h % 2 == 0:
                            return q8[:, base:base + 2, :]
                        return q8[:, base:base + 3:2, :]

                    o8 = p2out.tile([128, 4, 512], F8, tag="o8",
                                    name=f"o8_{qb}")
                    o8r = p2out.tile([128, 4, 512], F8, tag="o8r",
                                     name=f"o8r_{qb}")
                    for h in range(4):
                        nkt = 4 * (qb + 1)
                        npair = nkt // 2
                        ps_o = ps_op.tile([128, 512], F32, tag="pv")
                        dacc = p2acc.tile([128, 1024], F16, tag="dacc")

                        def emit_pv(exp_pair, pk, npair, ps_o=ps_o, h=h):
                            for j in (0, 1):
                                kt = 2 * pk + j
                                nc.tensor.matmul(
                                    ps_o[:],
                                    v_t[:, kt, 128 * h:128 * (h + 1)],
                                    exp_pair[:, 512 * j:512 * (j + 1)],
                                    start=(kt == 0), stop=(kt == 2 * npair - 1),
                                )

                        pend = []   # (exp pair tile, pk) one pair behind
                        for pk in range(npair):
                            ps_s = ps_sp.tile([128, 1024], F32, tag="scores")
                            for j in (0, 1):
                                kt = 2 * pk + j
                                ks = slice(128 * kt, 128 * (kt + 1))
                                delta = 128 * kt - 512 * qb
                                diag = delta >= 0
                                half = ps_s[:, 512 * j:512 * (j + 1)]
                                nc.tensor.matmul(
                                    half, kT8_t[:, h, :, ks], q8_mov(h),
                                    start=True, stop=not diag,
                                    perf_mode=DR_MODE,
                                )
                                if diag:
                                    nc.tensor.matmul(
                                        half, id_t[:],
                                        mask_t[:, :, 384 - delta:896 - delta],
                                        start=False, stop=True,
                                        perf_mode=DR_MODE,
                                    )
                            exp_t = p2exp.tile([128, 1024], F16, tag="exp")
                            nc.scalar.activation(
                                exp_t[:], ps_s[:], AF.Exp, scale=EXP_SCALE)
                            # two alternating accumulators halve the serial
                            # add-chain latency on DVE
                            half = dacc[:, 512 * (pk % 2):512 * (pk % 2) + 512]
                            if pk < 2:
                                nc.vector.tensor_add(
                                    half, exp_t[:, 0:512], exp_t[:, 512:1024])
                            else:
                                nc.vector.tensor_add(
                                    half, half, exp_t[:, 0:512])
                                nc.vector.tensor_add(
                                    half, half, exp_t[:, 512:1024])
                            pend.append((exp_t, pk))
                            if len(pend) > 2:
                                emit_pv(*pend.pop(0), npair)
                        for e in pend:
                            emit_pv(*e, npair)
                        dfold = p2acc1.tile([128, 512], F16, tag="dfold")
                        if npair > 1:
                            nc.vector.tensor_add(
                                dfold[:], dacc[:, 0:512], dacc[:, 512:1024])
                        else:
                            nc.vector.tensor_copy(dfold[:], dacc[:, 0:512])
                        dred = p2acc1.tile([128, 512], F32, tag="dred")
                        nc.gpsimd.partition_all_reduce(
                            dred[:], dfold[:], 128, bass_isa.ReduceOp.add)
                        rsb = p2tmp.tile([128, 512], F32, tag="rsbd")
                        nc.vector.reciprocal(rsb[:], dred[:])
                        o16 = p2tmp.tile([128, 512], F16, tag="o16")
                        nc.vector.tensor_mul(o16[:], ps_o[:], rsb[:])
                        nc.vector.tensor_copy(o8[:, h, :], o16[:])
                        nc.vector.tensor_sub(o8r[:, h, :], o16[:], o8[:, h, :])
                        # interleave W_o d-tiles of the previous q-block so
                        # the in-order PE stream has fill work during this
                        # block's exp-latency stalls
                        if prev_out is not None:
                            emit_wo(prev_out, prev_cs,
                                    range(4 * h, 4 * (h + 1)))
                        # interleave the next block's q projections mid-stream
                        # instead of serializing them at the block boundary
                        if h == 1 and 2 <= qb + 1 < n_sb:
                            q8_next = compute_q8(qb + 1)
                    prev_out, prev_cs = (o8, o8r), cs
                emit_wo(prev_out, prev_cs, alt=True)
            p2w_stack.close()
            persist_stack.close()

    nc.compile()
    return nc


def host_prep(inputs, S=S_FULL):
    """Build the 8 per-core input maps from the full problem inputs."""
    FP8 = ml_dtypes.float8_e4m3

    def to8(a):
        return np.ascontiguousarray(a).astype(FP8)

    def split8(a, scale):
        hi = (a * scale).astype(FP8)
        lo = (a * scale - hi.astype(np.float32)).astype(FP8)
        return hi, lo

    x = np.asarray(inputs["x"], np.float32)
    cosT = np.asarray(inputs["rope_cos"], np.float32).T
    sinT = np.asarray(inputs["rope_sin"], np.float32).T
    c4 = np.concatenate([cosT, cosT, cosT, cosT], 0) * SQ8
    s4 = np.concatenate([-sinT, sinT, -sinT, sinT], 0) * SQ8
    c4 = np.ascontiguousarray(c4).astype(np.float16)
    s4 = np.ascontiguousarray(s4).astype(np.float16)
    qw = np.asarray(inputs["q_norm_w"], np.float32)
    kvw = np.asarray(inputs["kv_norm_w"], np.float32)
    W_uq = np.asarray(inputs["W_uq"], np.float32) * qw[:, None]
    W_qr = np.asarray(inputs["W_qr"], np.float32) * qw[:, None]
    W_uk = np.asarray(inputs["W_uk"], np.float32) * kvw[:, None]
    W_kr = np.asarray(inputs["W_kr"], np.float32) * kvw[:, None]
    W_uv = np.asarray(inputs["W_uv"], np.float32) * kvw[:, None]
    W_o = np.asarray(inputs["W_o"], np.float32)
    W_dq = np.asarray(inputs["W_dq"], np.float32)
    W_dkv = np.asarray(inputs["W_dkv"], np.float32)

    def packw(a):
        # [D, C] -> [128, NKT*C]: partition-major device layout
        return np.ascontiguousarray(
            a.reshape(NKT, 128, -1).transpose(1, 0, 2).reshape(128, -1))

    wdq8, wdq8r = (packw(w) for w in split8(W_dq, BW))
    wdkv8, wdkv8r = (packw(w) for w in split8(W_dkv, BW))
    wo8_full, wo8r_full = split8(W_o, BWO)
    wuq8, wuq8r = split8(W_uq, BW)
    wqr8, wqr8r = split8(W_qr, BW)
    wuk8, wuk8r = split8(W_uk, BW)
    wkr8, wkr8r = split8(W_kr, BW)
    wuv8, wuv8r = split8(W_uv, BW)

    # mask table: plane 0 = {0, -240} causal pattern, plane 1 = 0
    cgrid = np.arange(896)[None, :] - 384
    igrid = np.arange(128)[:, None]
    mask8 = np.zeros((128, 2, 896), np.float32)
    mask8[:, 0, :] = np.where(cgrid >= igrid, 0.0, -240.0)
    mask8 = mask8.astype(FP8)
    id8 = np.zeros((128, 2, 128), np.float32)
    id8[:, 0, :] = MASK_ID * np.eye(128, dtype=np.float32)
    id8 = id8.astype(FP8)
    ones8 = np.ones((128, 1), np.float32).astype(FP8)

    in_maps = []
    for c in range(NCORES):
        b, g = c // 4, c % 4
        hs = slice(4 * g * DN, 4 * (g + 1) * DN)
        hr = slice(4 * g * DRR, 4 * (g + 1) * DRR)
        xT = np.ascontiguousarray(x[b].T)
        x8, x8r = split8(xT, AX)

        def pack(a):
            # [D, 512] -> [128, NKT*512]: partition-major device layout
            return np.ascontiguousarray(
                a.reshape(NKT, 128, -1).transpose(1, 0, 2).reshape(128, -1))

        def packs(a):
            # [D, S] -> [n_sb, 128, NKT*512]
            return np.ascontiguousarray(np.stack(
                [pack(a[:, 512 * c:512 * (c + 1)]) for c in range(S // 512)]))

        in_maps.append(dict(
            x8=packs(x8), x8r=packs(x8r),
            xkv8=pack(x8[:, 512 * g:512 * (g + 1)]),
            xkv8r=pack(x8r[:, 512 * g:512 * (g + 1)]),
            W_dq8=wdq8, W_dq8r=wdq8r,
            W_dkv8=wdkv8, W_dkv8r=wdkv8r,
            Wuq8=np.ascontiguousarray(wuq8[:, hs]),
            Wuq8r=np.ascontiguousarray(wuq8r[:, hs]),
            Wqr8=np.ascontiguousarray(wqr8[:, hr]),
            Wqr8r=np.ascontiguousarray(wqr8r[:, hr]),
            Wuk8=np.ascontiguousarray(wuk8[:, hs]),
            Wuk8r=np.ascontiguousarray(wuk8r[:, hs]),
            Wkr8=np.ascontiguousarray(wkr8[:, hr]),
            Wkr8r=np.ascontiguousarray(wkr8r[:, hr]),
            Wuv8=np.ascontiguousarray(wuv8[:, hs]),
            Wuv8r=np.ascontiguousarray(wuv8r[:, hs]),
            Wo8=np.ascontiguousarray(wo8_full[512 * g:512 * (g + 1), :]),
            Wo8r=np.ascontiguousarray(wo8r_full[512 * g:512 * (g + 1), :]),
            c4=c4, s4=s4, mask8=mask8, id8=id8, ones8=ones8,
        ))
    return in_maps


_NC_CACHE = {}


def kernel(**inputs) -> np.ndarray:
    S = np.asarray(inputs["x"]).shape[1]
    if S not in _NC_CACHE:
        _NC_CACHE[S] = build_nc(S)
    nc = _NC_CACHE[S]
    in_maps = host_prep(inputs, S)
    res = run_bass_kernel_spmd(nc, in_maps, core_ids=list(range(NCORES)))
    y = np.empty((B, S, D), np.float32)
    for b in range(B):
        acc = res.results[4 * b]["yT"].astype(np.float32)
        for g in range(1, 4):
            acc = acc + res.results[4 * b + g]["yT"].astype(np.float32)
        y[b] = acc.T
    return y
